# revision 23
# baseline (speedup 1.0000x reference)
"""DiffVolumeV2 Trainium2 kernel, v9 (DVE + PE hybrid).

out[b,c,d,h,x] = left[b,c,h,x] - right[b,c,h, clip(4x - d + 1, 0, Wr-1)]
with B=4, C=32, H=80, Wl=160, Wr=640, D=48.  10240 independent (b,c,h)
rows sharded contiguously across 8 cores; 10 tiles of 128 partitions.

Phase-plane decomposition (d = 4q+s; plane slots permuted so slot s holds
phase r_s = s^1, making every access affine over (s2, s1, q, x)):

    out[(4q+2*s2+s1)*WL + x] = left[x] - plane_slot[2*s2+s1][13 - s2 - q + x]

Work split per steady tile (95-98us total, v8 all-DVE was 98.7-99.7):
  * DVE: plane slots 0-2 via 3-free-dim InstTensorTensor (S3S3D3), two
    q-halves, plus the residual subtracts of the split pipeline.
  * PE: plane slot 3 via f32r identity matmuls.  f32r keeps 12 explicit
    mantissa bits and every reader of an f32r-tagged tile rounds on read,
    so an EXACT subtract is built from a 3-way split: Act Copy->f32r
    (= RNr) + DVE residual, twice; six accumulating 480-col matmuls per
    3q chunk then compute (r - l) in PSUM (left pieces are stored
    negated); the Act drain to bf16 applies scale=-1.  Reconstruction is
    bit-exact (verified on hw).  ~221ns per 480-col matmul after ramp.
  * Act: single-instruction 3-dim deinterleaves, split copies batched
    over tile pairs, one paired drain per q-half from [128,1024]
    double-bank PSUM tiles (one matmul still targets a single bank).
  * Ordering: splits for tile t+1/t+2 are emitted AFTER tile t's
    subtracts so the DVE chain never stalls on the Act ping-pong;
    deints run two tiles ahead.  DVE window ~76us (all-DVE floor 80.0),
    zero steady-state gaps.
  * IO: tile-0 right quartered on ring A + halved on ring B with tile-1's
    right as B's first transfer; tiles 3-9 inputs via SWDGE gated behind
    the head; outputs alternate the two HWDGE rings (2 d-halves/tile,
    tapered last tile).

Measured dead-ends (do not re-try): SWDGE output DMAs (2-ring rotation
beats 3-way with SWDGE by ~3.7us); pre-loop split ops at the head of the
DVE chain (+12us stall); flat [128,4096] PSUM tile (dep tracking
serializes PE behind Act); Act-queue DMA issues before the deints;
finer head DMA pieces (~0.7us issue + ~1.5us ring latency each).
Run-to-run noise is ~±1.5us for this variant.
"""

import numpy as np
from concourse import bacc, bass, tile
from concourse.bass_utils import run_bass_kernel_spmd
from concourse.tile_rust import add_dep_helper
import concourse.mybir as mybir

F32R = mybir.dt.float32r
F32 = mybir.dt.float32

B, C, H, WL, WR, D = 4, 32, 80, 160, 640, 48
N_CORES = 8
R = B * C * H            # 10240 rows
RPC = R // N_CORES       # 1280 rows per core
P = 128
TILES = RPC // P         # 10
PPAD = 13
PW = PPAD + WL           # 173
PLW = 4 * PW
TAPER = [(0, 3), (3, 3), (6, 3), (9, 2), (11, 1)]

_cached = None


def _build() -> bass.Bass:
    nc = bacc.Bacc()
    left_p = nc.declare_dram_parameter("left", [RPC, WL], mybir.dt.float32, isOutput=False)
    right_p = nc.declare_dram_parameter("right", [RPC, WR], mybir.dt.float32, isOutput=False)
    out_p = nc.declare_dram_parameter("out", [RPC, D, WL], mybir.dt.bfloat16, isOutput=True)
    id_p = nc.declare_dram_parameter("ident", [P, P], mybir.dt.float32, isOutput=False)
    out_flat = out_p[:].rearrange("r d x -> r (d x)")

    def ap(t, off, dims):
        return bass.AP(t.tensor, t.offset + off, [list(t.ap[0])] + dims)

    chains = {}

    def order(key, inst):
        prev = chains.get(key)
        if prev is not None:
            add_dep_helper(inst.ins, prev.ins, sync=False,
                           reason=f"{key} program order")
        chains[key] = inst
        return inst

    def tensor_tensor(out, in0, in1):
        eng = nc.vector
        return eng.add_instruction(
            mybir.InstTensorTensor(
                name=eng.bass.get_next_instruction_name(),
                op=mybir.AluOpType.subtract,
                ins=[eng.lower_ap(in0), eng.lower_ap(in1)],
                outs=[eng.lower_ap(out)],
            )
        )

    with tile.TileContext(nc) as tc:
        with tc.tile_pool(name="inp", bufs=1) as inp_pool, \
             tc.tile_pool(name="ot", bufs=3) as ot_pool, \
             tc.psum_pool(name="ps", bufs=1) as psum_pool:
            rt_all = inp_pool.tile([P, TILES * WR], mybir.dt.float32)
            lt_all = inp_pool.tile([P, TILES * WL], mybir.dt.float32)
            planes = inp_pool.tile([P, TILES * PLW], mybir.dt.float32)
            # PE path (plane slot 3): f32r split pieces, slot-3 rows only
            p3h = inp_pool.tile([P, TILES * PW], F32R)
            p3m = inp_pool.tile([P, TILES * PW], F32R)
            p3l = inp_pool.tile([P, TILES * PW], F32R)
            p3d1 = inp_pool.tile([P, TILES * PW], F32)
            p3d2 = inp_pool.tile([P, TILES * PW], F32)
            lh = inp_pool.tile([P, TILES * WL], F32R)
            lm = inp_pool.tile([P, TILES * WL], F32R)
            ll = inp_pool.tile([P, TILES * WL], F32R)
            ld1 = inp_pool.tile([P, TILES * WL], F32)
            ld2 = inp_pool.tile([P, TILES * WL], F32)
            idt = inp_pool.tile([P, P], F32R)
            psb = [psum_pool.tile([P, 1024], F32, name=f"psb{i}")
                   for i in range(4)]
            idr = bass.AP(idt.tensor, idt.offset, [list(idt.ap[0]), [1, P]])

            ringA, ringB = nc.sync, nc.scalar
            HQ = WR // 4     # 160-element quarter of a right row

            def load_right(eng, t, off, n):
                return eng.dma_start(
                    out=ap(rt_all, t * WR + off, [[1, n]]),
                    in_=bass.AP(right_p[:].tensor, t * P * WR + off,
                                [[WR, P], [1, n]]))

            def load_left(eng, t):
                eng.dma_start(
                    out=ap(lt_all, t * WL, [[1, WL]]),
                    in_=bass.AP(left_p[:].tensor, t * P * WL, [[WL, P], [1, WL]]))

            # Tile 0: right quarters 1,2 + left on ring A (deint of the
            # first quarter gates the first subtract), right half 2 on B.
            load_right(ringA, 0, 0, HQ)
            load_left(ringA, 0)
            load_right(ringA, 0, HQ, HQ)
            rh2 = load_right(ringB, 0, 2 * HQ, 2 * HQ)
            load_right(ringB, 1, 0, WR)
            ringA.dma_start(out=idt[:, :], in_=id_p[:, :].bitcast(F32R))
            load_left(ringA, 1)
            load_right(ringA, 2, 0, WR)
            load_left(ringB, 2)

            # Tiles 1-9 via the GpSimd SWDGE queue (desc-gen off the
            # ring-issuing queues, which carry the output stream).
            def load_group(t0, nt):
                first = nc.gpsimd.dma_start(
                    out=ap(rt_all, t0 * WR, [[WR, nt], [1, WR]]),
                    in_=bass.AP(right_p[:].tensor, t0 * P * WR,
                                [[WR, P], [WR * P, nt], [1, WR]]))
                nc.gpsimd.dma_start(
                    out=ap(lt_all, t0 * WL, [[WL, nt], [1, WL]]),
                    in_=bass.AP(left_p[:].tensor, t0 * P * WL,
                                [[WL, P], [WL * P, nt], [1, WL]]))
                return first

            g1 = load_group(3, 3)
            load_group(6, 4)
            add_dep_helper(g1.ins, rh2.ins, sync=True,
                           reason="keep head DMA engines clear of bulk input")

            Copy = mybir.ActivationFunctionType.Copy

            def deint(t, xoff, nx):
                # plane_slot[2*s2+s1][PPAD+xoff+u] = right[4*(xoff+u) + 2*s2+1-s1]
                order("act", nc.scalar.activation(
                    ap(planes, t * PLW + PPAD + xoff,
                       [[2 * PW, 2], [PW, 2], [1, nx]]),
                    ap(rt_all, t * WR + 4 * xoff + 1,
                       [[2, 2], [-1, 2], [4, nx]]),
                    Copy))

            def pad(t):
                order("act", nc.scalar.activation(
                    ap(planes, t * PLW, [[PW, 4], [1, PPAD]]),
                    ap(rt_all, t * WR, [[0, 4], [0, PPAD]]),
                    Copy))

            def sub(ot, po, lt_off, s2, q0, nq, xoff=0, nx=WL):
                return order("dve", tensor_tensor(
                    ap(ot, (4 * q0 + 2 * s2) * WL + xoff,
                       [[WL, 2], [4 * WL, nq], [1, nx]]),
                    ap(lt_all, lt_off + xoff, [[0, 2], [0, nq], [1, nx]]),
                    ap(planes, po + s2 * (2 * PW - 1) + PPAD - q0 + xoff,
                       [[PW, 2], [-1, nq], [1, nx]])))

            def sub_slot(ot, po, lt_off, s, q0, nq, xoff=0, nx=WL):
                # single-slot subtract: rows d = 4q+s from plane slot s
                c_s = s // 2
                return order("dve", tensor_tensor(
                    ap(ot, (4 * q0 + s) * WL + xoff,
                       [[4 * WL, nq], [1, nx]]),
                    ap(lt_all, lt_off + xoff, [[0, nq], [1, nx]]),
                    ap(planes, po + s * PW + PPAD - c_s - q0 + xoff,
                       [[-1, nq], [1, nx]])))

            Ident = mybir.ActivationFunctionType.Copy

            def splits(t, nt=1):
                # slot-3 plane rows of tiles t..t+nt-1 -> f32r pieces
                # (h, m, l); left rows likewise.  nt=2 halves instruction
                # count (tile ranges are contiguous in every buffer).
                p3 = t * PLW + 3 * PW
                o3 = t * PW
                ol = t * WL
                d3 = [[PLW, nt], [1, PW]]
                s3 = [[PW, nt], [1, PW]]
                sl = [[WL, nt], [1, WL]]
                order("act", nc.scalar.activation(
                    ap(p3h, o3, s3), ap(planes, p3, d3), Copy))
                order("act", nc.scalar.activation(
                    ap(lh, ol, sl), ap(lt_all, ol, sl), Copy,
                    scale=-1.0))
                order("dve", nc.vector.scalar_tensor_tensor(
                    ap(p3d1, o3, s3), ap(planes, p3, d3), 0.0,
                    bass.AP(p3h.tensor, p3h.offset + o3,
                            [list(p3h.ap[0])] + s3).bitcast(F32),
                    op0=mybir.AluOpType.bypass, op1=mybir.AluOpType.subtract))
                order("dve", nc.vector.scalar_tensor_tensor(
                    ap(ld1, ol, sl), ap(lt_all, ol, sl), -1.0,
                    bass.AP(lh.tensor, lh.offset + ol,
                            [list(lh.ap[0])] + sl).bitcast(F32),
                    op0=mybir.AluOpType.mult, op1=mybir.AluOpType.subtract))
                order("act", nc.scalar.activation(
                    ap(p3m, o3, s3), ap(p3d1, o3, s3), Copy))
                order("act", nc.scalar.activation(
                    ap(lm, ol, sl), ap(ld1, ol, sl), Copy))
                order("dve", nc.vector.scalar_tensor_tensor(
                    ap(p3d2, o3, s3), ap(p3d1, o3, s3), 0.0,
                    bass.AP(p3m.tensor, p3m.offset + o3,
                            [list(p3m.ap[0])] + s3).bitcast(F32),
                    op0=mybir.AluOpType.bypass, op1=mybir.AluOpType.subtract))
                order("dve", nc.vector.scalar_tensor_tensor(
                    ap(ld2, ol, sl), ap(ld1, ol, sl), 0.0,
                    bass.AP(lm.tensor, lm.offset + ol,
                            [list(lm.ap[0])] + sl).bitcast(F32),
                    op0=mybir.AluOpType.bypass, op1=mybir.AluOpType.subtract))
                order("act", nc.scalar.activation(
                    ap(p3l, o3, s3), ap(p3d2, o3, s3), Copy))
                order("act", nc.scalar.activation(
                    ap(ll, ol, sl), ap(ld2, ol, sl), Copy))

            pe_bank = [0]

            def pe_chunk(t, q0, nq):
                # slot-3 subtract for q0..q0+nq-1 via 6 accumulating f32r
                # identity matmuls:  PSUM = (lh+lm+ll) - (p3h+p3m+p3l)'
                # sign: stationary is +I; r pieces are accumulated positive
                # and the drain applies scale=-1:  out = -(r - l) = l - r.
                slot = pe_bank[0]
                pe_bank[0] += 1
                bank = psb[(slot // 2) % 4]
                cbase = (slot % 2) * 512
                cols = nq * WL
                o3 = t * PW + PPAD - 1 - q0
                ol = t * WL
                mms = []
                for i, (tile_, off, dims) in enumerate([
                        (p3h, o3, [[-1, nq], [1, WL]]),
                        (p3m, o3, [[-1, nq], [1, WL]]),
                        (p3l, o3, [[-1, nq], [1, WL]]),
                        (lh, ol, [[0, nq], [1, WL]]),
                        (lm, ol, [[0, nq], [1, WL]]),
                        (ll, ol, [[0, nq], [1, WL]]),
                ]):
                    mms.append(order("pe", nc.tensor.matmul(
                        bank[:, cbase:cbase + cols], idr,
                        bass.AP(tile_.tensor, tile_.offset + off,
                                [list(tile_.ap[0])] + dims),
                        start=(i == 0), stop=(i == 5))))
                return bank, cbase, mms[-1]

            def pe_drain(ot, bank, cbase, mm_last, q0, nq):
                # PSUM -> bf16 rows d = 4q+3, negated (l - r)
                dr = order("act", nc.scalar.activation(
                    ap(ot, (4 * q0 + 3) * WL, [[4 * WL, nq], [1, WL]]),
                    bank[:, cbase:cbase + nq * WL], Ident, scale=-1.0))
                add_dep_helper(dr.ins, mm_last.ins, sync=True,
                               reason="drain after accumulation")
                return dr

            def pe_drain2(ot, bank, mm_last, q0):
                # both 3q chunks (cols 0-479 and 512-991) of one double-bank
                # tile -> rows d = 4q+3 .. 4(q+5)+3, negated
                dr = order("act", nc.scalar.activation(
                    ap(ot, (4 * q0 + 3) * WL,
                       [[12 * WL, 2], [4 * WL, 3], [1, WL]]),
                    bass.AP(bank.tensor, bank.offset,
                            [list(bank.ap[0]), [512, 2], [WL, 3], [1, WL]]),
                    Ident, scale=-1.0))
                add_dep_helper(dr.ins, mm_last.ins, sync=True,
                               reason="drain after accumulation")
                return dr

            ring_i = [0]

            def out_dma(r0, ot, c0, cw):
                eng = ringA if ring_i[0] % 2 == 0 else ringB
                ring_i[0] += 1
                eng.dma_start(out=out_flat[r0:r0 + P, c0:c0 + cw],
                              in_=ot[:, c0:c0 + cw])

            # tile 0 head (all-DVE, x-pieces); tile 1 prelude before the loop
            pad(0)
            deint(0, 0, WL // 4)
            deint(0, WL // 4, WL // 4)
            deint(0, WL // 2, WL // 2)
            deint(1, 0, WL)
            pad(1)

            for t in range(TILES):
                r0 = t * P
                po = t * PLW
                ot = ot_pool.tile([P, D * WL], mybir.dt.bfloat16,
                                  name=f"ot{t}", tag="ot")
                drains = []
                if t == 0:
                    for xoff, nx in [(0, 40), (40, 40), (80, 80)]:
                        for s2 in range(2):
                            sub(ot, po, 0, s2, 0, 12, xoff=xoff, nx=nx)
                elif t < TILES - 1:
                    for qh in range(2):
                        sub(ot, po, t * WL, 0, 6 * qh, 6)
                        sub_slot(ot, po, t * WL, 2, 6 * qh, 6)
                        for c in range(2):
                            q0 = 6 * qh + 3 * c
                            bank, cbase, mm = pe_chunk(t, q0, 3)
                            drains.append((bank, cbase, mm, q0, 3))
                else:
                    for q0, nq in TAPER:
                        sub(ot, po, t * WL, 0, q0, nq)
                        sub_slot(ot, po, t * WL, 2, q0, nq)
                        if q0 < 9:
                            bank, cbase, mm = pe_chunk(t, q0, nq)
                            drains.append((bank, cbase, mm, q0, nq))
                        else:
                            sub_slot(ot, po, t * WL, 3, q0, nq)

                if t + 2 < TILES:
                    deint(t + 2, 0, WL)
                    pad(t + 2)
                if t == 0:
                    splits(1)
                if t % 2 == 1 and t + 2 < TILES:
                    splits(t + 1, 2)

                if t == 0:
                    out_dma(r0, ot, 0, 24 * WL)
                    out_dma(r0, ot, 24 * WL, 24 * WL)
                elif t < TILES - 1:
                    for qh in range(2):
                        bank, _, _, q0, _ = drains[2 * qh]
                        _, _, mm1, _, _ = drains[2 * qh + 1]
                        pe_drain2(ot, bank, mm1, q0)
                        out_dma(r0, ot, qh * 24 * WL, 24 * WL)
                else:
                    di = 0
                    for q0, nq in TAPER:
                        if q0 < 9:
                            bank, cbase, mm, dq0, dnq = drains[di]
                            di += 1
                            pe_drain(ot, bank, cbase, mm, dq0, dnq)
                        out_dma(r0, ot, 4 * q0 * WL, 4 * nq * WL)

    nc.finalize()
    return nc


def _run(left_feature, right_feature, trace=False, **trace_kw):
    global _cached
    left = np.ascontiguousarray(np.asarray(left_feature, dtype=np.float32).reshape(R, WL))
    right = np.ascontiguousarray(np.asarray(right_feature, dtype=np.float32).reshape(R, WR))
    if _cached is None:
        _cached = _build()
    nc = _cached
    ident = np.eye(P, dtype=np.float32)
    in_maps = [
        {"left": left[i * RPC:(i + 1) * RPC],
         "right": right[i * RPC:(i + 1) * RPC],
         "ident": ident}
        for i in range(N_CORES)
    ]
    res = run_bass_kernel_spmd(nc, in_maps, list(range(N_CORES)), trace=trace, **trace_kw)
    shards = [np.asarray(res.results[i]["out"]) for i in range(N_CORES)]
    full = np.concatenate(shards, axis=0).reshape(B, C, H, D, WL).transpose(0, 1, 3, 2, 4)
    return np.ascontiguousarray(full, dtype=np.float32), res


def kernel(left_feature, right_feature, max_disp=48, **_ignored):
    assert int(max_disp) == D
    out, _ = _run(left_feature, right_feature, trace=False)
    return out
